# revision 2
# baseline (speedup 1.0000x reference)
"""Trainium2 Bass kernel for nn_DetectionLoss (YOLO-style detection loss).

Pure data-parallel over batch: 8 cores x 4096 samples.

Per-core decomposition (everything per-sample, samples on partitions):
  loss_sum = 0.5 * sum_all softplus(po)                                (dense)
           + sum_assigned [ sp(-po) - 0.5*sp(po) ]                     (dense, mask)
           + 5 * sum_assigned smoothL1(pb - t)                         (dense)
           + 2 * sum_assigned cw * (logsumexp(pc) - pc[lbl])           (dense)
  total    = loss_sum / max(num_pos, 1)                                (host)

The sparse->dense connection uses the GPSIMD `local_scatter` custom op:
each partition independently scatters its targets' channel values
(mask, class-weight one-hot x3, tx, ty, tw, th as fp16) into dense
per-cell grids (Qc*98 cells per partition).  Duplicate cell assignments
are pre-resolved on DVE ("is there a later valid target with the same
cell" - matches jax scatter last-write-wins); losers/invalid targets get
index -1 which local_scatter ignores.  softplus(x) = -ln(sigmoid(-x)).
Each core emits per-partition partial sums; the host combines.
"""
import sys

sys.path.insert(0, "/opt/trn_rl_repo")

import numpy as np

import concourse.bass as bass
import concourse.bacc as bacc
import concourse.tile as tile
from concourse import mybir
from concourse.bass_utils import run_bass_kernel_spmd

F32 = mybir.dt.float32
F16 = mybir.dt.float16
I32 = mybir.dt.int32
I16 = mybir.dt.int16
BF16 = mybir.dt.bfloat16
ALU = mybir.AluOpType
ACTF = mybir.ActivationFunctionType
AX = mybir.AxisListType

G = 7
A = 2
C = 3
NCELL = G * G * A  # 98
ROW = 5 + C        # 8
M = 20
P = 128
N_CORES = 8
L_COORD, L_OBJ, L_NOOBJ, L_CLS = 5.0, 1.0, 0.5, 2.0

ANCHORS = np.array([[0.971, 1.7338], [3.4579, 5.1653]], dtype=np.float32)
CLASS_WEIGHTS = np.array([1.0, 4.9, 4.8], dtype=np.float32)

NPART = 16  # partials columns per half


def _ap(t, offset_delta, dims):
    """Custom AP over tile/AP t: keep partition dim, replace free dims."""
    base = t[:] if not isinstance(t, bass.AP) else t
    return bass.AP(base.tensor, base.offset + offset_delta, [base.ap[0]] + dims)


def build_program(Q, halves=2, repeat=0):
    """One-core SPMD program. B_core = 128*Q samples."""
    Bc = P * Q
    assert Q % halves == 0
    Qc = Q // halves          # sample-groups per half (per partition)
    QM = Qc * M               # targets per partition per half
    ND = Qc * NCELL           # dense cells per partition per half
    assert ND * 32 < 2 ** 16  # local_scatter num_elems limit
    nc = bacc.Bacc("TRN2", target_bir_lowering=False)

    preds = nc.dram_tensor("preds", [Bc * NCELL, ROW], F32, kind="ExternalInput")
    boxes = nc.dram_tensor("boxes", [Bc, M, 4], F32, kind="ExternalInput")
    labels = nc.dram_tensor("labels", [Bc, M], I32, kind="ExternalInput")
    nobj = nc.dram_tensor("nobj", [Bc], I32, kind="ExternalInput")
    out_part = nc.dram_tensor("partials", [P, NPART * halves], F32,
                              kind="ExternalOutput")

    a0w, a0h = float(ANCHORS[0, 0]), float(ANCHORS[0, 1])
    a1w, a1h = float(ANCHORS[1, 0]), float(ANCHORS[1, 1])
    lw0 = float(np.log(np.float32(a0w) + np.float32(1e-6)))
    lw1 = float(np.log(np.float32(a1w) + np.float32(1e-6)))
    lh0 = float(np.log(np.float32(a0h) + np.float32(1e-6)))
    lh1 = float(np.log(np.float32(a1h) + np.float32(1e-6)))
    w0, w1, w2 = [float(x) for x in CLASS_WEIGHTS]

    V = nc.vector
    S = nc.scalar
    GP = nc.gpsimd

    boxes_r = boxes[:].rearrange("(p q) m c -> p (q m c)", p=P)
    labels_r = labels[:].rearrange("(p q) m -> p (q m)", p=P)
    nobj_r = nobj[:].rearrange("(p q) -> p q", p=P)
    preds_r = preds[:].rearrange("(p r) h -> p (r h)", p=P)

    with tile.TileContext(nc) as tc:
        with (
            tc.tile_pool(name="const", bufs=1) as const,
            tc.tile_pool(name="work", bufs=1) as work,
        ):
            def ct(name, shape, dtype=F32):
                return const.tile(shape, dtype, name=name, tag=name)

            def mk(name, shape, dtype=F32, bufs=1, pool=None):
                del bufs
                return (pool or work).tile(shape, dtype, name=name, tag=name,
                                           bufs=1)

            # ---------------- constants ----------------
            iota_m_i = ct("iota_m_i", [P, QM], I32)
            GP.iota(iota_m_i[:], pattern=[[0, Qc], [1, M]], base=0,
                    channel_multiplier=0)
            iota_m = ct("iota_m", [P, QM])
            V.tensor_copy(iota_m[:], iota_m_i[:])
            junk = ct("junk", [P, QM])
            V.tensor_scalar_add(junk[:], iota_m[:], 100.0)
            # NUT[m, m'] = 1.0 if m' <= m else 0.0 (m-major layout)
            nut_i = ct("nut_i", [P, M * M], I32)
            GP.iota(nut_i[:], pattern=[[-1, M], [1, M]], base=0,
                    channel_multiplier=0)
            nut = ct("nut", [P, M * M], BF16)
            V.tensor_scalar(nut[:], nut_i[:], 0, None, op0=ALU.is_le)
            # q*NCELL per (q, m): dense cell base within the partition
            q98_i = ct("q98_i", [P, QM], I32)
            GP.iota(q98_i[:], pattern=[[NCELL, Qc], [0, M]], base=0,
                    channel_multiplier=0)
            q98 = ct("q98", [P, QM])
            V.tensor_copy(q98[:], q98_i[:])
            ones16 = ct("ones16", [P, QM], F16)
            V.memset(ones16[:], 1.0)
            cneg1 = ct("cneg1", [P, 1])
            V.memset(cneg1[:], -1.0)

            partials = ct("partials", [P, NPART * 2])
            V.memset(partials[:], 0.0)

            import contextlib
            loop_ctx = (tc.For_i(0, repeat, 1,
                                 hint_engines=(mybir.EngineType.DVE,))
                        if repeat else contextlib.nullcontext())

            # ---------------- per-half pipeline ----------------
            with loop_ctx:
              for h in range(2):
                  def col(i):
                      return partials[:, h * NPART + i:h * NPART + i + 1]

                  cS = h * Qc * M
                  bS = h * Qc * M * 4

                  # ---- loads ----
                  Tb = mk("Tb", [P, QM * 4])
                  nc.sync.dma_start(out=Tb[:], in_=boxes_r[:, bS:bS + QM * 4])
                  Tl_i = mk("Tl_i", [P, QM], I32)
                  nc.sync.dma_start(out=Tl_i[:], in_=labels_r[:, cS:cS + QM])
                  Tn_i = mk("Tn_i", [P, Qc], I32)
                  nc.sync.dma_start(out=Tn_i[:],
                                    in_=nobj_r[:, h * Qc:(h + 1) * Qc])
                  # full predictions for this half, cast to bf16 during DMA
                  PR = mk("PR", [P, ND * ROW], BF16)
                  GP.dma_start(out=PR[:],
                               in_=preds_r[:, h * ND * ROW:(h + 1) * ND * ROW])
                  po_v = _ap(PR, 0, [[ROW, ND]])
                  pc_v = _ap(PR, 5, [[ROW, ND], [1, C]])

                  lblf = mk("lblf", [P, QM], bufs=2)
                  V.tensor_copy(lblf[:], Tl_i[:])
                  nobjf = mk("nobjf", [P, Qc], bufs=2)
                  V.tensor_copy(nobjf[:], Tn_i[:])

                  x1 = _ap(Tb, 0, [[4, QM]])
                  y1 = _ap(Tb, 1, [[4, QM]])
                  x2 = _ap(Tb, 2, [[4, QM]])
                  y2 = _ap(Tb, 3, [[4, QM]])

                  # ---- per-target quantities ----
                  CXG = mk("CXG", [P, QM], bufs=2)
                  V.tensor_tensor(CXG[:], x1, x2, op=ALU.add)
                  V.tensor_scalar_mul(CXG[:], CXG[:], 0.5 * G)
                  CYG = mk("CYG", [P, QM], bufs=2)
                  V.tensor_tensor(CYG[:], y1, y2, op=ALU.add)
                  V.tensor_scalar_mul(CYG[:], CYG[:], 0.5 * G)

                  # floor via compare chain (values in [0, 7))
                  GJ = mk("GJ", [P, QM], bufs=2)
                  V.tensor_scalar(GJ[:], CXG[:], 1.0, None, op0=ALU.is_ge)
                  for k in range(2, G):
                      V.scalar_tensor_tensor(GJ[:], CXG[:], float(k), GJ[:],
                                             op0=ALU.is_ge, op1=ALU.add)
                  GI = mk("GI", [P, QM], bufs=2)
                  V.tensor_scalar(GI[:], CYG[:], 1.0, None, op0=ALU.is_ge)
                  for k in range(2, G):
                      V.scalar_tensor_tensor(GI[:], CYG[:], float(k), GI[:],
                                             op0=ALU.is_ge, op1=ALU.add)

                  # tx, ty (fp16 contiguous, ready for scatter)
                  TX = mk("TX", [P, QM], F16, bufs=2)
                  V.tensor_tensor(TX[:], CXG[:], GJ[:], op=ALU.subtract)
                  TY = mk("TY", [P, QM], F16, bufs=2)
                  V.tensor_tensor(TY[:], CYG[:], GI[:], op=ALU.subtract)

                  WG = mk("WG", [P, QM], bufs=2)
                  V.tensor_tensor(WG[:], x2, x1, op=ALU.subtract)
                  V.tensor_scalar_mul(WG[:], WG[:], float(G))
                  HG = mk("HG", [P, QM], bufs=2)
                  V.tensor_tensor(HG[:], y2, y1, op=ALU.subtract)
                  V.tensor_scalar_mul(HG[:], HG[:], float(G))

                  VALID = mk("VALID", [P, QM], bufs=2)
                  V.tensor_tensor(VALID[:], _ap(nobjf, 0, [[1, Qc], [0, M]]),
                                  iota_m[:], op=ALU.is_gt)

                  AR = mk("AR", [P, QM], bufs=2)
                  V.tensor_tensor(AR[:], WG[:], HG[:], op=ALU.mult)
                  T1 = mk("T1", [P, QM], bufs=2)
                  T2 = mk("T2", [P, QM], bufs=2)
                  # anchor 0
                  V.tensor_scalar_min(T1[:], WG[:], a0w)
                  V.tensor_scalar_min(T2[:], HG[:], a0h)
                  I0 = mk("I0", [P, QM], bufs=2)
                  V.tensor_tensor(I0[:], T1[:], T2[:], op=ALU.mult)
                  U0 = mk("U0", [P, QM], bufs=2)
                  V.tensor_scalar_add(U0[:], AR[:], a0w * a0h + 1e-6)
                  V.tensor_tensor(U0[:], U0[:], I0[:], op=ALU.subtract)
                  # anchor 1
                  V.tensor_scalar_min(T1[:], WG[:], a1w)
                  V.tensor_scalar_min(T2[:], HG[:], a1h)
                  I1 = mk("I1", [P, QM], bufs=2)
                  V.tensor_tensor(I1[:], T1[:], T2[:], op=ALU.mult)
                  U1 = mk("U1", [P, QM], bufs=2)
                  V.tensor_scalar_add(U1[:], AR[:], a1w * a1h + 1e-6)
                  V.tensor_tensor(U1[:], U1[:], I1[:], op=ALU.subtract)
                  # argmax via cross-multiply (strict > matches first-max)
                  V.tensor_tensor(T1[:], I1[:], U0[:], op=ALU.mult)
                  V.tensor_tensor(T2[:], I0[:], U1[:], op=ALU.mult)
                  BEST = mk("BEST", [P, QM], bufs=2)
                  V.tensor_tensor(BEST[:], T1[:], T2[:], op=ALU.is_gt)

                  # tw/th (fp16 contiguous)
                  V.tensor_scalar(T1[:], BEST[:], lw1 - lw0, lw0,
                                  op0=ALU.mult, op1=ALU.add)
                  V.tensor_scalar_max(T2[:], WG[:], 0.01)
                  LN1 = mk("LN1", [P, QM], bufs=2)
                  S.activation(LN1[:], T2[:], ACTF.Ln)
                  TW = mk("TW", [P, QM], F16, bufs=2)
                  V.tensor_tensor(TW[:], LN1[:], T1[:], op=ALU.subtract)
                  V.tensor_scalar(T1[:], BEST[:], lh1 - lh0, lh0,
                                  op0=ALU.mult, op1=ALU.add)
                  V.tensor_scalar_max(T2[:], HG[:], 0.01)
                  LN2 = mk("LN2", [P, QM], bufs=2)
                  S.activation(LN2[:], T2[:], ACTF.Ln)
                  TH = mk("TH", [P, QM], F16, bufs=2)
                  V.tensor_tensor(TH[:], LN2[:], T1[:], op=ALU.subtract)

                  FLAT = mk("FLAT", [P, QM], bufs=2)
                  V.scalar_tensor_tensor(FLAT[:], GI[:], float(G), GJ[:],
                                         op0=ALU.mult, op1=ALU.add)
                  V.scalar_tensor_tensor(FLAT[:], FLAT[:], float(A), BEST[:],
                                         op0=ALU.mult, op1=ALU.add)

                  FENC = mk("FENC", [P, QM], bufs=2)
                  V.tensor_copy(FENC[:], junk[:])
                  VALID_I = mk("VALID_I", [P, QM], I32, bufs=2)
                  V.tensor_copy(VALID_I[:], VALID[:])
                  V.copy_predicated(FENC[:], VALID_I[:], FLAT[:])

                  # ---- owner detection (last valid wins) ----
                  EQ = mk("EQ", [P, Qc, M, M], BF16, bufs=1)
                  fencA = _ap(FENC, 0, [[M, Qc], [1, M], [0, M]])
                  fencB = _ap(FENC, 0, [[M, Qc], [0, M], [1, M]])
                  V.tensor_tensor(EQ[:], fencA, fencB, op=ALU.is_equal)
                  V.scalar_tensor_tensor(
                      EQ[:], EQ[:], 1.0,
                      _ap(nut, 0, [[0, Qc], [M, M], [1, M]]),
                      op0=ALU.mult, op1=ALU.subtract)
                  DUP = mk("DUP", [P, QM], bufs=2)
                  V.tensor_reduce(DUP[:], EQ[:], axis=AX.X, op=ALU.max)
                  OWNER = mk("OWNER", [P, QM], bufs=2)
                  V.scalar_tensor_tensor(OWNER[:], DUP[:], 0.0, VALID[:],
                                         op0=ALU.is_le, op1=ALU.mult,
                                         accum_out=col(14))

                  # ---- scatter indices: owner ? q*98+flat : -1 ----
                  CIDX = mk("CIDX", [P, QM], bufs=2)
                  V.tensor_tensor(CIDX[:], FLAT[:], q98[:], op=ALU.add)
                  V.tensor_scalar_add(CIDX[:], CIDX[:], 1.0)
                  V.tensor_tensor(CIDX[:], CIDX[:], OWNER[:], op=ALU.mult)
                  V.tensor_scalar_add(CIDX[:], CIDX[:], -1.0)
                  IDX16 = mk("IDX16", [P, QM], I16, bufs=2)
                  V.tensor_copy(IDX16[:], CIDX[:])

                  # ---- class-weight one-hot channels (fp16) ----
                  WOH = []
                  for c in range(C):
                      wc = mk(f"WOH{c}", [P, QM], F16, bufs=2)
                      V.tensor_scalar(wc[:], lblf[:], float(c),
                                      [w0, w1, w2][c],
                                      op0=ALU.is_equal, op1=ALU.mult)
                      WOH.append(wc)

                  # ---- local scatters into dense grids ----
                  def scat(name, data_t):
                      g = mk(name, [P, ND], F16)
                      GP.local_scatter(out_ap=g[:], data_ap=data_t[:],
                                       idxs_ap=IDX16[:], channels=P,
                                       num_elems=ND, num_idxs=QM)
                      return g

                  MKD = scat("MKD", ones16)
                  W0D = scat("W0D", WOH[0])
                  W1D = scat("W1D", WOH[1])
                  W2D = scat("W2D", WOH[2])
                  TXD = scat("TXD", TX)
                  TYD = scat("TYD", TY)
                  TWD = scat("TWD", TW)
                  THD = scat("THD", TH)

                  # ---- dense: obj / noobj (Exp/Ln only, no table switch) ----
                  # sp(po) = ln(1 + e^po); col0 = +sum sp(po) over all cells.
                  # objt = sp(-po) - 0.5*sp(po) = 0.5*sp(po) - po
                  EXPD = mk("EXPD", [P, ND])
                  S.activation(EXPD[:], po_v, ACTF.Exp)
                  V.tensor_scalar_add(EXPD[:], EXPD[:], 1.0)
                  SPD = mk("SPD", [P, ND], BF16)
                  S.activation(SPD[:], EXPD[:], ACTF.Ln, accum_out=col(0))
                  OBD = mk("OBD", [P, ND])
                  V.scalar_tensor_tensor(OBD[:], SPD[:], L_NOOBJ, po_v,
                                         op0=ALU.mult, op1=ALU.subtract)
                  V.scalar_tensor_tensor(OBD[:], OBD[:], 1.0, MKD[:],
                                         op0=ALU.mult, op1=ALU.mult,
                                         accum_out=col(1))

                  # ---- dense: smooth L1 via sl1 = 0.5 d^2 - 0.5 relu(|d|-1)^2
                  # squares+sums run on ACT with fused accumulation; DVE only
                  # does mask-mult, subtract, and the sign-bit abs.
                  PBC = mk("PBC", [P, ND], F16)
                  DD = mk("DD", [P, ND], F16)
                  RLD = mk("RLD", [P, ND], F16)
                  SQS = mk("SQS", [P, ND], F16)
                  for ci, TD in enumerate([TXD, TYD, TWD, THD]):
                      pb_c = _ap(PR, 1 + ci, [[ROW, ND]])
                      # masked pred: 0 at unassigned cells (TD is 0 there too)
                      V.tensor_tensor(PBC[:], pb_c, MKD[:], op=ALU.mult)
                      V.tensor_tensor(DD[:], PBC[:], TD[:], op=ALU.subtract)
                      ddi = DD[:].bitcast(I16)
                      V.tensor_scalar(ddi, ddi, 0x7FFF, None,
                                      op0=ALU.bitwise_and)
                      # col(2+ci): sum |d|^2 ; col(6+ci): sum relu(|d|-1)^2
                      S.activation(SQS[:], DD[:], ACTF.Square,
                                   accum_out=col(2 + ci))
                      S.activation(RLD[:], DD[:], ACTF.Relu, bias=cneg1[:])
                      S.activation(SQS[:], RLD[:], ACTF.Square,
                                   accum_out=col(6 + ci))

                  # ---- dense: weighted cross entropy ----
                  EZD = mk("EZD", [P, ND, C], BF16, bufs=2)
                  S.activation(EZD[:], pc_v, ACTF.Exp)
                  ZD = mk("ZD", [P, ND], bufs=2)
                  e0 = _ap(EZD, 0, [[C, ND]])
                  e1 = _ap(EZD, 1, [[C, ND]])
                  e2 = _ap(EZD, 2, [[C, ND]])
                  V.tensor_tensor(ZD[:], e0, e1, op=ALU.add)
                  V.tensor_tensor(ZD[:], ZD[:], e2, op=ALU.add)
                  LZD = mk("LZD", [P, ND], BF16, bufs=2)
                  S.activation(LZD[:], ZD[:], ACTF.Ln)
                  CWD = mk("CWD", [P, ND], bufs=2)
                  V.tensor_tensor(CWD[:], W0D[:], W1D[:], op=ALU.add)
                  V.tensor_tensor(CWD[:], CWD[:], W2D[:], op=ALU.add)
                  V.scalar_tensor_tensor(CWD[:], CWD[:], 1.0, LZD[:],
                                         op0=ALU.mult, op1=ALU.mult,
                                         accum_out=col(10))
                  LGT = mk("LGT", [P, ND], bufs=2)
                  for c, WD in enumerate([W0D, W1D, W2D]):
                      pc_c = _ap(PR, 5 + c, [[ROW, ND]])
                      V.scalar_tensor_tensor(LGT[:], WD[:], 1.0, pc_c,
                                             op0=ALU.mult, op1=ALU.mult,
                                             accum_out=col(11 + c))

            nc.sync.dma_start(out=out_part[:], in_=partials[:])

    nc.finalize()
    return nc


_CACHE = {}


def _get_program(Q):
    if Q not in _CACHE:
        _CACHE[Q] = build_program(Q)
    return _CACHE[Q]


def shard_inputs(predictions, target_boxes, target_labels, num_objs):
    B = predictions.shape[0]
    Bc = B // N_CORES
    preds = np.ascontiguousarray(predictions, dtype=np.float32).reshape(
        N_CORES, Bc * NCELL, ROW)
    boxes = np.ascontiguousarray(target_boxes, dtype=np.float32).reshape(
        N_CORES, Bc, M, 4)
    labels = np.ascontiguousarray(target_labels, dtype=np.int32).reshape(
        N_CORES, Bc, M)
    nobj = np.ascontiguousarray(num_objs, dtype=np.int32).reshape(N_CORES, Bc)
    return [
        dict(preds=preds[i], boxes=boxes[i], labels=labels[i], nobj=nobj[i])
        for i in range(N_CORES)
    ]


def combine_partials(parts, halves=2):
    """parts: list of (P, NPART*halves) arrays."""
    s = np.zeros(NPART, np.float64)
    for p in parts:
        p = p.astype(np.float64)
        for h in range(halves):
            s += p[:, h * NPART:(h + 1) * NPART].sum(axis=0)
    sp_all = s[0]              # sum sp(po) over all cells
    obj_a = s[1]               # sum mask*(sp(-po) - 0.5 sp(po))
    sl1 = 0.5 * (s[2] + s[3] + s[4] + s[5] - s[6] - s[7] - s[8] - s[9])
    ce_lz = s[10]
    ce_logit = s[11] + s[12] + s[13]
    npos = s[14]
    loss_sum = (L_NOOBJ * sp_all + obj_a + L_COORD * sl1
                + L_CLS * (ce_lz - ce_logit))
    total = loss_sum / max(npos, 1.0)
    return np.float32(total)


LAST_EXEC_NS = None
LAST_RESULTS = None


def kernel(predictions, target_boxes, target_labels, num_objs,
           anchors=None, class_weights=None, **_):
    global LAST_EXEC_NS, LAST_RESULTS
    B = predictions.shape[0]
    Q = B // (N_CORES * P)
    nc = _get_program(Q)
    in_maps = shard_inputs(predictions, target_boxes, target_labels, num_objs)
    res = run_bass_kernel_spmd(nc, in_maps, core_ids=list(range(N_CORES)))
    LAST_EXEC_NS = res.exec_time_ns
    LAST_RESULTS = res
    return combine_partials([r["partials"] for r in res.results])



# revision 18
# speedup vs baseline: 1.5971x; 1.5971x over previous
"""Trainium2 Bass kernel for nn_DetectionLoss (YOLO-style detection loss).

Pure data-parallel over batch: 8 cores x 4096 samples (128 partitions x 32
samples each).

v4 design notes:
  - Per-target pipeline runs once over the full per-core batch with (x,y)
    pairs packed into [P, 1280] ops where possible; the linear chains
    (scale, floor-rounding, log-input clamping) run on the otherwise-idle
    ACT engine via Copy/Relu/Ln with scale/bias.
  - floor(x) = rne(x - 0.5) using the +/-1.5*2^23 magic add (exact f32,
    identical on HW and interpreter; x in [0,7), only exact integer is 0
    where rne(-0.5) = -0 = floor).
  - Duplicate-target resolution relies on the GPSIMD local_scatter being
    last-write-wins per partition (verified bit-identical against an
    explicit O(M^2) dedup pass on hardware); invalid/duplicate handling
    reduces to a validity gate.
  - Dense phase: 8 DMA chunks (f32 HWDGE, triple-buffered, overlapped with
    compute); per chunk, GPSIMD scatters fill mask / class-weight / packed
    box-target grids and the dense reductions run with fused accumulators
    (accum_out partial columns), double-buffered so chunks pipeline.
  - All activations used (exp/ln/square/abs/relu/copy) are steered into
    the single natural_log_exp_and_others table set -> one table load.

Per-core partial sums are combined on the host.
"""
import sys

sys.path.insert(0, "/opt/trn_rl_repo")

import numpy as np

import concourse.bass as bass
import concourse.bacc as bacc
import concourse.tile as tile
from concourse import mybir
from concourse.bass_utils import run_bass_kernel_spmd

# The ACT table-load pass alternates between the exp-only and ln-only
# table sets (2 loads x 8 chunks = ~20us of ACT_TABLE_LOAD).  Every
# activation this kernel uses lives in the single
# "natural_log_exp_and_others" set, so steer the pass there by
# advertising exp/ln only from that set.  Set order (and therefore
# act_func_set_id numbering) is preserved.
_ORIG_GAT = bacc.get_activation_tables


def _gat_combined(arch):
    t = {k: set(v) for k, v in _ORIG_GAT(arch).items()}
    if "natural_log_exp_and_others" in t:
        for k, v in t.items():
            if k != "natural_log_exp_and_others":
                v.discard(mybir.ActivationFunctionType.Exp)
                v.discard(mybir.ActivationFunctionType.Ln)
    return t


bacc.get_activation_tables = _gat_combined

F32 = mybir.dt.float32
F16 = mybir.dt.float16
I32 = mybir.dt.int32
I16 = mybir.dt.int16
BF16 = mybir.dt.bfloat16
ALU = mybir.AluOpType
ACTF = mybir.ActivationFunctionType
AX = mybir.AxisListType

G = 7
A = 2
C = 3
NCELL = G * G * A  # 98
ROW = 5 + C        # 8
M = 20
P = 128
N_CORES = 8
L_COORD, L_OBJ, L_NOOBJ, L_CLS = 5.0, 1.0, 0.5, 2.0

ANCHORS = np.array([[0.971, 1.7338], [3.4579, 5.1653]], dtype=np.float32)
CLASS_WEIGHTS = np.array([1.0, 4.9, 4.8], dtype=np.float32)
# f16-exact values of the class weights (cw grid is stored f16)
CW_F16 = [float(np.float16(np.float32(w))) for w in CLASS_WEIGHTS]

PCOL = 9  # partial columns per chunk:
#   0 sp, 1 obj, 2 d2, 3 r2, 4 cwlz, 5 npos, 6-8 ind


def _ap(t, offset_delta, dims):
    """Custom AP over tile/AP t: keep partition dim, replace free dims."""
    base = t[:] if not isinstance(t, bass.AP) else t
    return bass.AP(base.tensor, base.offset + offset_delta, [base.ap[0]] + dims)


def build_program(Q, dedup=False):
    """One-core SPMD program. B_core = 128*Q samples."""
    Bc = P * Q
    NCH = 8                  # DMA / scatter chunks
    assert Q % NCH == 0
    Qq = Q // NCH            # samples per partition per chunk (4)
    NDq = Qq * NCELL         # dense cells per partition per chunk (392)
    NDe = NDq * 4            # packed coord grid size per chunk (1568)
    ND2 = NDq * 2            # dense cells per pair (784)
    QM = Q * M               # targets per partition (640)
    QM2 = QM * 2
    assert NDq * 32 < 2 ** 16 and NDe * 32 < 2 ** 16
    NCOL = PCOL * NCH

    nc = bacc.Bacc("TRN2", target_bir_lowering=False)

    preds = nc.dram_tensor("preds", [Bc * NCELL, ROW], F32, kind="ExternalInput")
    boxes = nc.dram_tensor("boxes", [Bc, M, 4], F32, kind="ExternalInput")
    labels = nc.dram_tensor("labels", [Bc, M], I32, kind="ExternalInput")
    nobj = nc.dram_tensor("nobj", [Bc], I32, kind="ExternalInput")
    out_part = nc.dram_tensor("partials", [P, NCOL], F32, kind="ExternalOutput")

    a0w, a0h = float(ANCHORS[0, 0]), float(ANCHORS[0, 1])
    a1w, a1h = float(ANCHORS[1, 0]), float(ANCHORS[1, 1])
    lw0 = float(np.log(np.float32(a0w) + np.float32(1e-6)))
    lw1 = float(np.log(np.float32(a1w) + np.float32(1e-6)))
    lh0 = float(np.log(np.float32(a0h) + np.float32(1e-6)))
    lh1 = float(np.log(np.float32(a1h) + np.float32(1e-6)))
    MAGIC = float(np.float32(8388608.0) * 1.5)

    V = nc.vector
    S = nc.scalar
    GP = nc.gpsimd

    boxes_r = boxes[:].rearrange("(p q) m c -> p (q m c)", p=P)
    labels_r = labels[:].rearrange("(p q) m -> p (q m)", p=P)
    nobj_r = nobj[:].rearrange("(p q) -> p q", p=P)
    preds_r = preds[:].rearrange("(p r) h -> p (r h)", p=P)

    with tile.TileContext(nc) as tc:
        with (
            tc.tile_pool(name="const", bufs=1) as const,
            tc.tile_pool(name="work", bufs=1) as work,
        ):
            def ct(name, shape, dtype=F32):
                return const.tile(shape, dtype, name=name, tag=name)

            def sc(name, value):
                t = ct(name, [P, 1])
                V.memset(t[:], value)
                return t

            def mk(name, shape, dtype=F32, bufs=1):
                return work.tile(shape, dtype, name=name, tag=name, bufs=bufs)

            # ---------------- constants ----------------
            itmp = ct("itmp", [P, QM], I32)
            iota_m = ct("iota_m", [P, QM])
            GP.iota(itmp[:], pattern=[[0, Q], [1, M]], base=0,
                    channel_multiplier=0)
            V.tensor_copy(iota_m[:], itmp[:])
            # chunk-local dense cell base per (q, m), pre-shifted by +1:
            # 98 * (q mod Qq) + 1
            q98p1 = ct("q98p1", [P, QM])
            GP.iota(itmp[:], pattern=[[0, NCH], [NCELL, Qq], [0, M]], base=1,
                    channel_multiplier=0)
            V.tensor_copy(q98p1[:], itmp[:])
            # c - 4 for packed-coord index build
            iot4 = ct("iot4", [P, 4])
            GP.iota(itmp[:, 0:4], pattern=[[0, 1], [1, 4]], base=-4,
                    channel_multiplier=0)
            V.tensor_copy(iot4[:], itmp[:, 0:4])
            if dedup:
                nut_i = ct("nut_i", [P, M * M], I32)
                GP.iota(nut_i[:], pattern=[[-1, M], [1, M]], base=0,
                        channel_multiplier=0)
                nut = ct("nut", [P, M * M], BF16)
                V.tensor_scalar(nut[:], nut_i[:], 0, None, op0=ALU.is_le)
            ones16 = ct("ones16", [P, QM], F16)
            V.memset(ones16[:], 1.0)
            anch0 = ct("anch0", [P, 2])
            V.memset(anch0[:, 0:1], a0w)
            V.memset(anch0[:, 1:2], a0h)
            anch1 = ct("anch1", [P, 2])
            V.memset(anch1[:, 0:1], a1w)
            V.memset(anch1[:, 1:2], a1h)
            cneg1 = sc("cneg1", -1.0)
            cn001 = sc("cn001", -0.01)
            c001 = sc("c001", 0.01)

            partials = ct("partials", [P, NCOL])
            V.memset(partials[:], 0.0)

            # ---------------- input loads ----------------
            Tb = mk("Tb", [P, QM * 4])
            nc.sync.dma_start(out=Tb[:], in_=boxes_r[:])
            Tl_i = mk("Tl_i", [P, QM], I32)
            nc.sync.dma_start(out=Tl_i[:], in_=labels_r[:])
            Tn_i = mk("Tn_i", [P, Q], I32)
            nc.sync.dma_start(out=Tn_i[:], in_=nobj_r[:])

            # prefetch predictions (f32, HWDGE), triple-buffered chunks
            PRs = []
            for qt in range(NCH):
                PR = mk("PR", [P, NDq * ROW], bufs=3)
                nc.sync.dma_start(
                    out=PR[:],
                    in_=preds_r[:, qt * NDq * ROW:(qt + 1) * NDq * ROW])
                PRs.append(PR)

            # ---------------- per-target pipeline (full Q) ----------------
            lblf = mk("lblf", [P, QM])
            V.tensor_copy(lblf[:], Tl_i[:])
            nobjf = mk("nobjf", [P, Q])
            V.tensor_copy(nobjf[:], Tn_i[:])

            # packed (x, y) views of the boxes: [t, {x|y}]
            XY1 = _ap(Tb, 0, [[4, QM], [1, 2]])
            XY2 = _ap(Tb, 2, [[4, QM], [1, 2]])

            # interleaved (tx, ty, tw, th) scatter payload
            TD = mk("TD", [P, QM * 4], F16)

            # centers*G and floor via ACT (Copy with scale/bias)
            SXY = mk("SXY", [P, QM2])
            V.tensor_tensor(SXY[:], XY1, XY2, op=ALU.add)
            CG = mk("CG", [P, QM2])
            S.activation(CG[:], SXY[:], ACTF.Copy, scale=0.5 * G)
            GIJ = mk("GIJ", [P, QM2])
            S.activation(GIJ[:], CG[:], ACTF.Copy, bias=-0.5)
            S.activation(GIJ[:], GIJ[:], ACTF.Copy, bias=MAGIC)
            S.activation(GIJ[:], GIJ[:], ACTF.Copy, bias=-MAGIC)
            V.tensor_tensor(_ap(TD, 0, [[4, QM], [1, 2]]), CG[:], GIJ[:],
                            op=ALU.subtract)

            # widths/heights*G
            SWH = mk("SXY", [P, QM2])
            V.tensor_tensor(SWH[:], XY2, XY1, op=ALU.subtract)
            WH = mk("WH", [P, QM2])
            S.activation(WH[:], SWH[:], ACTF.Copy, scale=float(G))

            VALID = mk("VALID", [P, QM])
            V.tensor_tensor(VALID[:], _ap(nobjf, 0, [[1, Q], [0, M]]),
                            iota_m[:], op=ALU.is_gt)

            # anchor argmax via cross-multiplied IoU compare
            AN0 = mk("AN0", [P, QM2])
            V.tensor_tensor(AN0[:], WH[:], _ap(anch0, 0, [[0, QM], [1, 2]]),
                            op=ALU.min)
            AN1 = mk("AN1", [P, QM2])
            V.tensor_tensor(AN1[:], WH[:], _ap(anch1, 0, [[0, QM], [1, 2]]),
                            op=ALU.min)
            I0 = mk("I0", [P, QM])
            V.tensor_tensor(I0[:], _ap(AN0, 0, [[2, QM]]),
                            _ap(AN0, 1, [[2, QM]]), op=ALU.mult)
            I1 = mk("I1", [P, QM])
            V.tensor_tensor(I1[:], _ap(AN1, 0, [[2, QM]]),
                            _ap(AN1, 1, [[2, QM]]), op=ALU.mult)
            AR = mk("AR", [P, QM])
            V.tensor_tensor(AR[:], _ap(WH, 0, [[2, QM]]),
                            _ap(WH, 1, [[2, QM]]), op=ALU.mult)
            U0 = mk("U0", [P, QM])
            V.scalar_tensor_tensor(U0[:], AR[:], a0w * a0h + 1e-6, I0[:],
                                   op0=ALU.add, op1=ALU.subtract)
            U1 = mk("U1", [P, QM])
            V.scalar_tensor_tensor(U1[:], AR[:], a1w * a1h + 1e-6, I1[:],
                                   op0=ALU.add, op1=ALU.subtract)
            T1 = mk("T1", [P, QM])
            V.tensor_tensor(T1[:], I1[:], U0[:], op=ALU.mult)
            T2 = mk("AR", [P, QM])
            V.tensor_tensor(T2[:], I0[:], U1[:], op=ALU.mult)
            BEST = mk("U0", [P, QM])
            V.tensor_tensor(BEST[:], T1[:], T2[:], op=ALU.is_gt)

            # tw/th = ln(max(wh, 0.01)) - ln(anchor+1e-6), anchor by BEST.
            # max via Relu+bias, packed Ln over the (w,h) pair.
            T1WH = mk("AN0", [P, QM2])
            S.activation(_ap(T1WH, 0, [[2, QM]]), BEST[:], ACTF.Copy,
                         scale=lw1 - lw0, bias=lw0)
            S.activation(_ap(T1WH, 1, [[2, QM]]), BEST[:], ACTF.Copy,
                         scale=lh1 - lh0, bias=lh0)
            REL = mk("CG", [P, QM2])
            S.activation(REL[:], WH[:], ACTF.Relu, bias=cn001[:])
            LNWH = mk("WH", [P, QM2])
            S.activation(LNWH[:], REL[:], ACTF.Ln, bias=c001[:])
            V.tensor_tensor(_ap(TD, 2, [[4, QM], [1, 2]]), LNWH[:], T1WH[:],
                            op=ALU.subtract)

            # flat cell index: (gi*7 + gj)*2 + best
            FLAT = mk("I0", [P, QM])
            V.scalar_tensor_tensor(FLAT[:], _ap(GIJ, 1, [[2, QM]]), float(G),
                                   _ap(GIJ, 0, [[2, QM]]),
                                   op0=ALU.mult, op1=ALU.add)
            V.scalar_tensor_tensor(FLAT[:], FLAT[:], float(A), BEST[:],
                                   op0=ALU.mult, op1=ALU.add)

            # class-weight payload (f16)
            CW1 = mk("I1", [P, QM])
            V.tensor_scalar(CW1[:], lblf[:], 1.0, float(CLASS_WEIGHTS[1]) - 1.0,
                            op0=ALU.is_equal, op1=ALU.mult)
            CW2 = mk("U1", [P, QM])
            V.tensor_scalar(CW2[:], lblf[:], 2.0, float(CLASS_WEIGHTS[2]) - 1.0,
                            op0=ALU.is_equal, op1=ALU.mult)
            V.tensor_tensor(CW1[:], CW1[:], CW2[:], op=ALU.add)
            CW16 = mk("CW16", [P, QM], F16)
            V.tensor_scalar_add(CW16[:], CW1[:], 1.0)

            if dedup:
                # explicit last-valid-wins duplicate resolution (debug path;
                # HW local_scatter already resolves duplicates this way)
                A0 = mk("A0", [P, QM])
                V.tensor_scalar_add(A0[:], FLAT[:], 1.0)
                VCID = mk("VCID", [P, QM], BF16)
                V.tensor_tensor(VCID[:], A0[:], VALID[:], op=ALU.mult)
                EQ = mk("EQ", [P, QM, M], BF16)
                fencA = _ap(VCID, 0, [[1, QM], [0, M]])
                fencB = _ap(VCID, 0, [[M, Q], [0, M], [1, M]])
                V.tensor_tensor(EQ[:], fencA, fencB, op=ALU.is_equal)
                V.scalar_tensor_tensor(
                    EQ[:], EQ[:], 1.0,
                    _ap(nut, 0, [[0, Q], [M, M], [1, M]]),
                    op0=ALU.mult, op1=ALU.subtract)
                DUP = mk("DUP", [P, QM])
                V.tensor_reduce(DUP[:], EQ[:], axis=AX.X, op=ALU.max)
                OWNER = mk("OWNER", [P, QM])
                V.scalar_tensor_tensor(OWNER[:], DUP[:], 0.0, VALID[:],
                                       op0=ALU.is_le, op1=ALU.mult)
            else:
                OWNER = VALID

            # scatter index tiles: chunk-local cell+1 gated to 0 for
            # invalid targets, then shifted to -1 / packed *4+c-4
            AQ = mk("T1", [P, QM])
            V.tensor_tensor(AQ[:], FLAT[:], q98p1[:], op=ALU.add)
            V.tensor_tensor(AQ[:], AQ[:], OWNER[:], op=ALU.mult)
            CQ16 = mk("CQ16", [P, QM], I16)
            V.tensor_scalar_add(CQ16[:], AQ[:], -1.0)
            IDX4 = mk("IDX4", [P, QM * 4], I16)
            V.scalar_tensor_tensor(IDX4[:], _ap(AQ, 0, [[1, QM], [0, 4]]), 4.0,
                                   _ap(iot4, 0, [[0, QM], [1, 4]]),
                                   op0=ALU.mult, op1=ALU.add)

            # ---------------- dense phase, per chunk ----------------
            for qt in range(NCH):
                def col(i):
                    return partials[:, qt * PCOL + i:qt * PCOL + i + 1]

                PR = PRs[qt]
                po_v = _ap(PR, 0, [[ROW, NDq]])
                pb_v = _ap(PR, 1, [[ROW, NDq], [1, 4]])
                pc_v = _ap(PR, 5, [[ROW, NDq], [1, 3]])
                tS = qt * Qq * M

                MKD = mk("MKD", [P, NDq], F16, bufs=2)
                GP.local_scatter(out_ap=MKD[:],
                                 data_ap=ones16[:, tS:tS + Qq * M],
                                 idxs_ap=CQ16[:, tS:tS + Qq * M], channels=P,
                                 num_elems=NDq, num_idxs=Qq * M)
                CWD = mk("CWD", [P, NDq], F16, bufs=2)
                GP.local_scatter(out_ap=CWD[:],
                                 data_ap=CW16[:, tS:tS + Qq * M],
                                 idxs_ap=CQ16[:, tS:tS + Qq * M], channels=P,
                                 num_elems=NDq, num_idxs=Qq * M)
                TD4 = mk("TD4", [P, NDe], F16, bufs=2)
                GP.local_scatter(
                    out_ap=TD4[:], data_ap=TD[:, tS * 4:(tS + Qq * M) * 4],
                    idxs_ap=IDX4[:, tS * 4:(tS + Qq * M) * 4], channels=P,
                    num_elems=NDe, num_idxs=Qq * M * 4)

                # sp(po) = ln(1 + exp(po)); col0 = sum sp over all cells
                EXPD = mk("EXPD", [P, NDq], BF16, bufs=2)
                S.activation(EXPD[:], po_v, ACTF.Exp)
                EZD = mk("EZD", [P, NDq, C], BF16, bufs=2)
                S.activation(EZD[:], pc_v, ACTF.Exp)
                SPD = mk("SPD", [P, NDq], BF16, bufs=2)
                S.activation(SPD[:], EXPD[:], ACTF.Ln, bias=1.0,
                             accum_out=col(0))
                OB = mk("OB", [P, NDq], BF16, bufs=2)
                V.scalar_tensor_tensor(OB[:], SPD[:], L_NOOBJ, po_v,
                                       op0=ALU.mult, op1=ALU.subtract)
                V.scalar_tensor_tensor(OB[:], OB[:], 1.0, MKD[:],
                                       op0=ALU.mult, op1=ALU.mult,
                                       accum_out=col(1))

                # smooth L1 on packed coords: d = pb*mk - t
                PB4 = mk("PB4", [P, NDe], F16, bufs=2)
                V.tensor_tensor(PB4[:], pb_v, _ap(MKD, 0, [[1, NDq], [0, 4]]),
                                op=ALU.mult)
                V.tensor_tensor(PB4[:], PB4[:], TD4[:], op=ALU.subtract)
                SQ = mk("SQ", [P, NDe], F16, bufs=2)
                S.activation(SQ[:], PB4[:], ACTF.Square, accum_out=col(2))
                AB = mk("AB", [P, NDe], F16, bufs=2)
                S.activation(AB[:], PB4[:], ACTF.Abs)
                S.activation(SQ[:], AB[:], ACTF.Relu, bias=cneg1[:])
                S.activation(AB[:], SQ[:], ACTF.Square, accum_out=col(3))

                # weighted cross entropy
                ZD = mk("ZD", [P, NDq], BF16, bufs=2)
                V.tensor_tensor(ZD[:], _ap(EZD, 0, [[C, NDq]]),
                                _ap(EZD, 1, [[C, NDq]]), op=ALU.add)
                V.tensor_tensor(ZD[:], ZD[:], _ap(EZD, 2, [[C, NDq]]),
                                op=ALU.add)
                LZD = mk("LZD", [P, NDq], BF16, bufs=2)
                S.activation(LZD[:], ZD[:], ACTF.Ln)
                DA = mk("DA", [P, NDq], BF16, bufs=2)
                V.scalar_tensor_tensor(DA[:], CWD[:], 1.0, LZD[:],
                                       op0=ALU.mult, op1=ALU.mult,
                                       accum_out=col(4))
                # num_pos = sum mask (mask is 0/1 so min(mask,1) == mask)
                V.tensor_scalar(OB[:], MKD[:], 1.0, None, op0=ALU.min,
                                op1=ALU.add, accum_out=col(5))
                # label-selected logits via (cw == w_c)
                for c in range(C):
                    V.scalar_tensor_tensor(
                        DA[:], CWD[:], CW_F16[c],
                        _ap(PR, 5 + c, [[ROW, NDq]]),
                        op0=ALU.is_equal, op1=ALU.mult,
                        accum_out=col(6 + c))

            nc.sync.dma_start(out=out_part[:], in_=partials[:])

    nc.finalize()
    return nc


_CACHE = {}


def _get_program(Q, dedup=False):
    key = (Q, dedup)
    if key not in _CACHE:
        _CACHE[key] = build_program(Q, dedup)
    return _CACHE[key]


def shard_inputs(predictions, target_boxes, target_labels, num_objs):
    B = predictions.shape[0]
    Bc = B // N_CORES
    preds = np.ascontiguousarray(predictions, dtype=np.float32).reshape(
        N_CORES, Bc * NCELL, ROW)
    boxes = np.ascontiguousarray(target_boxes, dtype=np.float32).reshape(
        N_CORES, Bc, M, 4)
    labels = np.ascontiguousarray(target_labels, dtype=np.int32).reshape(
        N_CORES, Bc, M)
    nobj = np.ascontiguousarray(num_objs, dtype=np.int32).reshape(N_CORES, Bc)
    return [
        dict(preds=preds[i], boxes=boxes[i], labels=labels[i], nobj=nobj[i])
        for i in range(N_CORES)
    ]


def combine_partials(parts):
    """parts: list of (P, PCOL*8) arrays."""
    sp_all = obj_a = d2 = r2 = cwlz = npos = 0.0
    pc = np.zeros(3, np.float64)
    for p in parts:
        p = p.astype(np.float64)
        for pp in range(8):
            q = p[:, pp * PCOL:(pp + 1) * PCOL].sum(axis=0)
            sp_all += q[0]
            obj_a += q[1]
            d2 += q[2]
            r2 += q[3]
            cwlz += q[4]
            npos += q[5]
            pc += q[6:9]
    sl1 = 0.5 * (d2 - r2)
    ce = cwlz - float(np.dot(CLASS_WEIGHTS.astype(np.float64), pc))
    loss_sum = (L_NOOBJ * sp_all + obj_a + L_COORD * sl1 + L_CLS * ce)
    total = loss_sum / max(npos, 1.0)
    return np.float32(total)


LAST_EXEC_NS = None
LAST_RESULTS = None


def kernel(predictions, target_boxes, target_labels, num_objs,
           anchors=None, class_weights=None, **_):
    global LAST_EXEC_NS, LAST_RESULTS
    import os
    B = predictions.shape[0]
    Q = B // (N_CORES * P)
    dedup = bool(os.environ.get("KERNEL_DEDUP"))
    nc = _get_program(Q, dedup)
    in_maps = shard_inputs(predictions, target_boxes, target_labels, num_objs)
    res = run_bass_kernel_spmd(nc, in_maps, core_ids=list(range(N_CORES)))
    LAST_EXEC_NS = res.exec_time_ns
    LAST_RESULTS = res
    return combine_partials([r["partials"] for r in res.results])
